# revision 34
# baseline (speedup 1.0000x reference)
"""Axial (frame-local) attention kernel for Trainium2, 8-core data-parallel.

Problem: x[4, 8192, 512] -> qkv proj -> per-(batch, head, frame) attention over
256-token frames (f=32 frames of 256 tokens in an 8192 sequence) -> out proj.

Sharding: pure data-parallel over (batch, half-sequence): core c handles
batch c//2, tokens (c%2)*4096 .. +4096 (16 whole frames). No collectives.

Per-core pipeline (chunks of 512 tokens):
  - load x chunk host-transposed as xT [dim, tok] (feature-major)
  - qT,kT = (w_qkv block)^T-matmul in [feat, tok] layout; v natural [tok, feat]
  - per (frame, head): sim^T = k q^T on PE -> exp on ScalarE (no max-subtract;
    logits are O(6) so fp32 exp is safe) -> ov = [v|1]^T p~ on PE produces both
    the unnormalized attention output AND the softmax denominator Z (row 64)
  - normalize: Z row -> SBUF partition 0 (ACT copy), 1/Z via the custom-DVE
    fast reciprocal (input MUST be at partition 0 - any partition offset
    reads garbage or wedges the core), GPSIMD partition-broadcast, then one
    merged DVE scalar_tensor_tensor per frame fuses PSUM read + mul + f32r
    cast for BOTH heads into a per-pair [64, 2, tok] tile; DMA shifts place
    the two head parities at ptile rows 0-63 / 64-127 for the projection
  - output projection from the transposed layout + bias, DMA out

Engine balance per chunk (PE ~20.5us): ACT = 8 exps + 8 Z-exits + 4 v-copies
+ 2 qk-copies ~= 18us; DVE = 6 qk-copies + 8 recips + 8 merged muls + 4 proj
biases ~= 18us; GPSIMD = 8 broadcasts ~= 8us. The PE queue is kept saturated:
each attention pair's exp latency is covered by interleaving the next chunk's
qkv slice (or the held previous projection for the final chunk) between the
sim and ov matmuls; the final chunk's own projection is injected per-frame as
soon as its columns complete. A run of junk warmup matmuls at t=0 ramps the
PE p-state (half clock until ~3us of continuous busy) while the first DMAs
land; weight DMAs are issued in first-use column order on the queues that
are idle early (vector/scalar), with w_out deferred past the x prefetches.

Matmul operands use float32r (single-pass fp32, ~tf32 precision, 2x faster
than fp32's LOW_HIGH two-pass mode; free dim >= 256 streams 1 row/cycle).
PSUM accumulation stays fp32.
"""

import sys
import types

import numpy as np

import concourse.tile as tile
from concourse import bacc, mybir
from concourse.bass import ts
from concourse.bass_utils import run_bass_kernel_spmd

F32 = mybir.dt.float32
F32R = mybir.dt.float32r
AF = mybir.ActivationFunctionType
ALU = mybir.AluOpType

# Model dims (hardcoded per problem spec)
B, SEQ, D = 4, 8192, 512
HEADS, DH = 8, 64
INNER = HEADS * DH  # 512
SCALE = DH ** -0.5
FRAME = 256  # n_sp = seq // f = 8192 // 32
N_CORES = 8
T = (B * SEQ) // N_CORES  # 4096 tokens per core
CHUNK = 512  # tokens per inner iteration
NCH = T // CHUNK  # 8
FPC = CHUNK // FRAME  # frames per chunk = 2
SPC = CHUNK // 128  # 128-token subtiles per chunk = 4

# matmul operand dtype: F32R (single-pass, ~tf32) or F32 (two-pass, exact)
MM_DT = F32R
# attention operand dtype. bf16 was re-tried with the saturated-PE schedule:
# 255.6us vs 254.0us for f32r and 9x the error — the ~210ns steady cadence of
# N=256 matmuls is dtype-independent, so f32r's accuracy is free.
AT_DT = F32R

def _install_ntff_hook():
    """The trimmed container's antenv lacks axon_hooks; inject it so
    run_bass_kernel_spmd(trace=True) can capture NTFF profiles."""
    if "antenv.axon_hooks" in sys.modules:
        return
    try:
        from trn_agent_boot.trn_boot import _ntff_profile_via_ctypes

        hook = _ntff_profile_via_ctypes("/opt/axon/libaxon_pjrt.so")
    except Exception:
        return
    mod = types.ModuleType("antenv.axon_hooks")
    mod._hook = hook
    mod.get_axon_ntff_profile_hook = lambda: mod._hook
    mod.set_axon_ntff_profile_hook = lambda h: setattr(mod, "_hook", h)
    sys.modules["antenv.axon_hooks"] = mod


def _pin_act_tables():
    """Keep Exp (the only table-backed ACT function used) pinned to one table
    set so the loader never reloads tables (~1.3us each) mid-kernel."""
    from concourse.hw_specs import get_activation_tables

    tabs = get_activation_tables(_pin_act_tables.arch)
    keep = "natural_log_exp_and_others"
    if keep not in tabs:
        return
    for name, fns in tabs.items():
        if name != keep:
            fns.discard(AF.Exp)
            fns.discard(AF.Ln)


# w_qkv column groups in first-use order: QK_ORDER consumes ptiles
# (4,5),(0,1),(6,7),(2,3) then v (cols 1024:1536)
W_COL_GROUPS = [(512, 768), (0, 512), (768, 1536)]
QK_ORDER = (4, 5, 0, 1, 6, 7, 2, 3)
# qk PSUM->SBUF copies run on ACT for these ptiles, DVE otherwise (2/6
# split balances the engines; see module docstring)
QK_ACT_COPY = (4, 0, 6)


def _build_body(nc, tc, ctx, x_ap, wqkv_ap, wout_ap, bout_ap, out_ap, n_chunks=NCH):
    mm_dt = MM_DT
    at_dt = AT_DT
    pconst = ctx.enter_context(tc.tile_pool(name="const", bufs=1))
    pxt = ctx.enter_context(tc.tile_pool(name="xt", bufs=8))
    pqk = ctx.enter_context(tc.tile_pool(name="qk", bufs=16))
    pvx = ctx.enter_context(tc.tile_pool(name="vx", bufs=6))
    ppt = ctx.enter_context(tc.tile_pool(name="pt", bufs=5))
    prz = ctx.enter_context(tc.tile_pool(name="rz", bufs=3))
    prb = ctx.enter_context(tc.tile_pool(name="rb", bufs=4))
    pot = ctx.enter_context(tc.tile_pool(name="ot", bufs=10))
    py = ctx.enter_context(tc.tile_pool(name="y", bufs=3))
    pwu = ctx.enter_context(tc.tile_pool(name="wu", bufs=1))
    pmm = ctx.enter_context(tc.tile_pool(name="mm", bufs=2, space="PSUM"))
    psim = ctx.enter_context(tc.tile_pool(name="sim", bufs=2, space="PSUM"))
    povp = ctx.enter_context(tc.tile_pool(name="ovp", bufs=2, space="PSUM"))

    def warmup():
        # Junk matmuls at t=0: keep the PE continuously busy from the first
        # cycle so its p-state ramps to full clock (half speed until ~3us of
        # uninterrupted execution) while the first x/weight DMAs land.
        # the memset is required (the tile framework rejects reads of
        # never-written tiles); it lives on the vector queue, which issues
        # no DMAs and has no other work before the attention phase
        wt = pwu.tile([128, CHUNK], mm_dt, tag="wu")
        nc.vector.memset(wt[:].bitcast(F32), 0.0)
        # dummy ACT ops: pull the ~1.3us activation-table loads (fired at
        # first use of each function class) into the startup DMA window
        # instead of the first attention pair's critical path
        wa = pwu.tile([1, 16], F32, tag="wuact")
        nc.scalar.copy(wa[:], wt[0:1, 0:16].bitcast(F32))
        nc.scalar.activation(wa[:], wt[0:1, 0:16].bitcast(F32), AF.Exp)
        for i in range(7):
            ps = pmm.tile([128, CHUNK], F32, tag="mm")
            nc.tensor.matmul(ps[:], wt[:, 0:128], wt[:], start=True, stop=True)

    w_kts = [
        pconst.tile([128, 3 * INNER], mm_dt, tag=f"wqkv{kt}", name=f"wqkv{kt}")
        for kt in range(4)
    ]
    wo_sb = pconst.tile([128, 4, D], mm_dt, tag="wout")
    b1 = pconst.tile([1, D], F32, tag="b1")
    bb = pconst.tile([128, D], F32, tag="bb")

    def load_consts():
        # Weight DMAs in first-use column order, round-robined over the
        # DMA-capable queues (scalar/gpsimd see no real work for ~25us;
        # sync starts after the x prefetches). Vector cannot issue DMAs.
        qeng = [nc.scalar, nc.gpsimd, nc.sync]
        qi = 0
        for lo, hi in W_COL_GROUPS:
            for kt in range(4):
                qeng[qi % 3].dma_start(
                    w_kts[kt][:, lo:hi],
                    wqkv_ap.bitcast(mm_dt)[kt * 128 : (kt + 1) * 128, lo:hi],
                )
                qi += 1

    def load_consts_late():
        # w_out/bias are first needed by proj(0) at ~35us; issue them after
        # the chunk-2 x prefetch so they don't crowd the startup HBM window
        nc.scalar.dma_start(
            wo_sb[:], wout_ap.bitcast(mm_dt).rearrange("(kt p) e -> p kt e", p=128)
        )
        nc.gpsimd.dma_start(b1[:], bout_ap.rearrange("(a d) -> a d", a=1))
        nc.gpsimd.partition_broadcast(bb[:], b1[:])

    def ld(ci):
        tb = ci * CHUNK
        # ---- load xT chunk: x arrives host-transposed [D, T], so the
        # feature-major tiles the matmuls need come straight off DMA ----
        xts = []
        for db in range(4):
            xt = pxt.tile([128, CHUNK], mm_dt, tag="xt", bufs=10)
            nc.sync.dma_start(
                xt[:],
                x_ap.bitcast(mm_dt)[db * 128 : (db + 1) * 128, tb : tb + CHUNK],
            )
            xts.append(xt)
        return xts

    def _qk_finish(ci, p, ps, qkd, qkod):
        qs = pqk.tile([128, CHUNK], at_dt, tag="qk", bufs=10)
        if p in QK_ACT_COPY:
            nc.scalar.copy(qs[:], ps[:])
        else:
            nc.vector.tensor_copy(qs[:], ps[:])
        qkd[p] = qs
        # odd heads live at partitions 64-127; matmul operands must sit
        # at base partition 0 (tile_position row 64 faults on this
        # runtime), so shift them down with SBUF->SBUF DMA right after
        # the cast (DMA is address-based)
        qo = pqk.tile([64, CHUNK], at_dt, tag="qko", name=f"qko{ci}_{p}", bufs=9)
        nc.sync.dma_start(qo[:], qs[64:128, :])
        qkod[p] = qo

    def make_slice_fills(ci, xts, sl, qkd, qkod):
        """qT/kT groups for 2 ptiles (slice sl of 4), split 6/2 around the
        ov matmuls: the second ptile's last two accumulations land AFTER
        the pair's ov matmuls so the ovp PSUM tiles rotate with more slack
        than the normalize-chain latency (z-exit->recip->bcast->STT
        ~3.3us; the PSUM pool has only 2 buffers). k-ptiles for quad 0
        first: attention's first sim matmuls need ptiles (4,5,0,1)."""
        st = {}

        def fa():
            pa, pb = QK_ORDER[2 * sl : 2 * sl + 2]
            ps = pmm.tile([128, CHUNK], F32, tag="mm")
            for kt in range(4):
                nc.tensor.matmul(
                    ps[:], w_kts[kt][:, ts(pa, 128)], xts[kt][:],
                    start=(kt == 0), stop=(kt == 3),
                )
            _qk_finish(ci, pa, ps, qkd, qkod)
            psb = pmm.tile([128, CHUNK], F32, tag="mm")
            for kt in range(2):
                nc.tensor.matmul(
                    psb[:], w_kts[kt][:, ts(pb, 128)], xts[kt][:],
                    start=(kt == 0), stop=False,
                )
            st["psb"], st["pb"] = psb, pb

        def fb():
            psb, pb = st["psb"], st["pb"]
            for kt in range(2, 4):
                nc.tensor.matmul(
                    psb[:], w_kts[kt][:, ts(pb, 128)], xts[kt][:],
                    start=False, stop=(kt == 3),
                )
            _qk_finish(ci, pb, psb, qkd, qkod)

        return fa, fb

    def qv_qk_slice(ci, xts, sl, qkd, qkod):
        fa, fb = make_slice_fills(ci, xts, sl, qkd, qkod)
        fa()
        fb()

    def qv_v(ci, xts):
        # ---- v natural [tok, feat] + ones column -> vext [128, h, 65] ----
        vexts = []
        for t in range(SPC):
            ps = pmm.tile([128, INNER], F32, tag="mm")
            for kt in range(4):
                nc.tensor.matmul(
                    ps[:],
                    xts[kt][:, ts(t, 128)],
                    w_kts[kt][:, 2 * INNER : 3 * INNER],
                    start=(kt == 0),
                    stop=(kt == 3),
                )
            vx = pvx.tile([128, HEADS, DH + 1], at_dt, tag="vx", bufs=5)
            nc.vector.memset(vx[:, :, DH : DH + 1].bitcast(F32), 1.0)
            nc.scalar.copy(
                vx[:, :, 0:DH], ps[:].rearrange("p (h d) -> p h d", h=HEADS)
            )
            vexts.append(vx)
        return vexts

    def qv(ci, xts):
        qkd = {}
        qkod = {}
        for sl in range(4):
            qv_qk_slice(ci, xts, sl, qkd, qkod)
        vexts = qv_v(ci, xts)
        return [qkd[p] for p in range(8)], [qkod[p] for p in range(8)], vexts

    def attn_alloc(ci):
        # otls: per-128-feature ptiles, the proj stationary. pos: per-pair
        # [64, parity, tok] staging written by the merged normalize STT;
        # DMA shifts route parity 0 -> otls rows 0-63, parity 1 -> 64-127.
        otls = [
            pot.tile([128, CHUNK], mm_dt, tag="ot", name=f"ot{ci}_{i}")
            for i in range(4)
        ]
        pos_ = [
            pot.tile([DH, 2, CHUNK], mm_dt, tag="po", name=f"po{ci}_{i}", bufs=6)
            for i in range(4)
        ]
        return otls, pos_

    def attn_pair(
        ci, st, otls, pos_, pr,
        fill=None, fill_b=None, fi_hook=None, shift_each_fi=False,
    ):
        qkts, qkos, vexts = st
        po = pos_[pr]
        # sim^T for both heads x both frames, per key-side 128-tok tile:
        # psum cols = (hp, fi) * FRAME
        pts = []
        for jt in range(2):
            sim = psim.tile([128, 4 * FRAME], F32, tag="sim")
            for hp in range(2):
                h = 2 * pr + hp
                if h % 2 == 0:
                    ck = qkts[4 + h // 2][0:64, :]
                    cq = qkts[h // 2][0:64, :]
                else:
                    ck = qkos[4 + h // 2][:]
                    cq = qkos[h // 2][:]
                for fi in range(FPC):
                    f0 = fi * FRAME
                    nc.tensor.matmul(
                        sim[:, ts(2 * hp + fi, FRAME)],
                        ck[:, f0 + jt * 128 : f0 + (jt + 1) * 128],
                        cq[:, f0 : f0 + FRAME],
                        start=True,
                        stop=True,
                    )
            pt = ppt.tile([128, 4 * FRAME], at_dt, tag="pt")
            nc.scalar.activation(pt[:], sim[:], AF.Exp, scale=SCALE)
            pts.append(pt)
        if fill is not None:
            # emitted between the sim and ov matmuls: the PE chews on
            # independent work (next chunk's qkv / held projection) while
            # the ACT exps drain
            fill()
        for fi in range(FPC):
            f0 = fi * FRAME
            ovp = povp.tile([DH + 1, 2 * FRAME], F32, tag="ovp")
            for hp in range(2):
                for jt in range(2):
                    nc.tensor.matmul(
                        ovp[:, ts(hp, FRAME)],
                        vexts[fi * 2 + jt][:, 2 * pr + hp, :],
                        pts[jt][:, ts(2 * hp + fi, FRAME)],
                        start=(jt == 0),
                        stop=(jt == 1),
                    )
            # softmax denominators for both heads sit in row 64 (the ones
            # column of vext): ACT copies them to partition 0, the custom
            # DVE reciprocal (offset-0 input only!) inverts, GPSIMD
            # broadcasts, one merged DVE STT normalizes both heads.
            zt = prz.tile([1, 2 * FRAME], F32, tag="z", bufs=3)
            nc.scalar.copy(zt[:], ovp[DH : DH + 1, :])
            rz = prz.tile([1, 2 * FRAME], F32, tag="rz", bufs=3)
            nc.vector.reciprocal_approx_fast(out=rz[:], in_=zt[:])
            zb = prb.tile([DH, 2 * FRAME], F32, tag="zb")
            nc.gpsimd.partition_broadcast(zb[:], rz[:])
            nc.vector.scalar_tensor_tensor(
                po[:, :, f0 : f0 + FRAME],
                ovp[0:DH, :].rearrange("p (a b) -> p a b", a=2),
                1.0,
                zb[:].rearrange("p (a b) -> p a b", a=2),
                op0=ALU.mult,
                op1=ALU.mult,
            )
            if shift_each_fi:
                for par in range(2):
                    nc.sync.dma_start(
                        otls[pr][par * DH : (par + 1) * DH, f0 : f0 + FRAME],
                        po[:, par, f0 : f0 + FRAME],
                    )
            elif fi == FPC - 1:
                for par in range(2):
                    nc.sync.dma_start(
                        otls[pr][par * DH : (par + 1) * DH, :], po[:, par, :]
                    )
            if fi_hook is not None:
                fi_hook(fi)
        if fill_b is not None:
            fill_b()

    def proj_sub(ci, otls, s):
        tb = ci * CHUNK
        ps = pmm.tile([128, D], F32, tag="mm")
        for p in range(4):
            nc.tensor.matmul(
                ps[:],
                otls[p][:, ts(s, 128)],
                wo_sb[:, p, :],
                start=(p == 0),
                stop=(p == 3),
            )
        y = py.tile([128, D], F32, tag="y", bufs=3)
        nc.vector.scalar_tensor_tensor(
            y[:], ps[:], 1.0, bb[:], op0=ALU.mult, op1=ALU.add
        )
        nc.sync.dma_start(out_ap[tb + s * 128 : tb + (s + 1) * 128, :], y[:])

    def proj(ci, otls):
        for s in range(SPC):
            proj_sub(ci, otls, s)

    # Software pipeline: chunk ci's attention pairs interleave the next
    # chunk's qkv slices (fill) so the PE never idles on exp latency; the
    # second-to-last chunk's projection is held and becomes the fill for
    # the final chunk's pairs, whose own projection is injected per-frame.
    warmup()
    lds = {0: ld(0)}
    load_consts()
    if n_chunks > 1:
        lds[1] = ld(1)
    st = qv(0, lds.pop(0))
    held = None
    for ci in range(n_chunks):
        last = ci == n_chunks - 1
        otls, pos_ = attn_alloc(ci)
        nxt_x = None
        if not last:
            if ci + 2 < n_chunks:
                lds[ci + 2] = ld(ci + 2)
            if ci == 0:
                load_consts_late()
            nxt_x = lds.pop(ci + 1)
        qkd, qkod = {}, {}
        for pr in range(4):
            fill = fill_b = None
            if nxt_x is not None:
                fill, fill_b = make_slice_fills(ci + 1, nxt_x, pr, qkd, qkod)
            elif held is not None:
                fill = lambda pr=pr: proj_sub(held[0], held[1], pr)
            fi_hook = None
            if last and pr == 3:
                def fi_hook(fi, otls=otls):
                    # final chunk: emit the projection for each frame as
                    # soon as all pairs' columns for it have shifted
                    proj_sub(ci, otls, 2 * fi)
                    proj_sub(ci, otls, 2 * fi + 1)
            attn_pair(
                ci, st, otls, pos_, pr,
                fill=fill, fill_b=fill_b, fi_hook=fi_hook, shift_each_fi=last,
            )
        if not last:
            vexts = qv_v(ci + 1, nxt_x)
            st = ([qkd[p] for p in range(8)], [qkod[p] for p in range(8)], vexts)
        if ci == n_chunks - 2 and n_chunks > 1:
            held = (ci, otls)
        elif not last:
            proj(ci, otls)


_CACHE = {}


def _get_nc(n_chunks=NCH):
    key = ("nc", n_chunks, str(MM_DT))
    if key in _CACHE:
        return _CACHE[key]
    from contextlib import ExitStack

    nc = bacc.Bacc("TRN2", target_bir_lowering=False, debug=False, num_devices=N_CORES)
    _pin_act_tables.arch = nc.m.arch
    _pin_act_tables()
    t_tok = n_chunks * CHUNK
    x_ap = nc.dram_tensor("x", [D, t_tok], F32, kind="ExternalInput").ap()
    wqkv_ap = nc.dram_tensor("w_qkv", [D, 3 * INNER], F32, kind="ExternalInput").ap()
    wout_ap = nc.dram_tensor("w_out", [INNER, D], F32, kind="ExternalInput").ap()
    bout_ap = nc.dram_tensor("b_out", [D], F32, kind="ExternalInput").ap()
    out_ap = nc.dram_tensor("out", [t_tok, D], F32, kind="ExternalOutput").ap()
    with tile.TileContext(nc) as tc:
        with ExitStack() as ctx:
            _build_body(
                nc, tc, ctx, x_ap, wqkv_ap, wout_ap, bout_ap, out_ap, n_chunks=n_chunks
            )
    nc.compile()
    _CACHE[key] = nc
    return nc


def _make_in_maps(x, w_qkv, w_out, b_out):
    x = np.ascontiguousarray(np.asarray(x, dtype=np.float32))
    w_qkv = np.ascontiguousarray(np.asarray(w_qkv, dtype=np.float32))
    w_out = np.ascontiguousarray(np.asarray(w_out, dtype=np.float32))
    b_out = np.ascontiguousarray(np.asarray(b_out, dtype=np.float32))
    assert x.shape == (B, SEQ, D), x.shape
    in_maps = []
    for c in range(N_CORES):
        b = c // 2
        t0 = (c % 2) * T
        in_maps.append(
            {
                "x": np.ascontiguousarray(x[b, t0 : t0 + T, :].T),
                "w_qkv": w_qkv,
                "w_out": w_out,
                "b_out": b_out,
            }
        )
    return in_maps


def _assemble(results):
    out = np.empty((B, SEQ, D), dtype=np.float32)
    for c in range(N_CORES):
        b = c // 2
        t0 = (c % 2) * T
        out[b, t0 : t0 + T, :] = results[c]["out"]
    return out


def run(x, w_qkv, w_out, b_out, f=32, trace=False):
    assert int(f) == 32, f"kernel hardcoded for f=32, got {f}"
    _install_ntff_hook()
    nc = _get_nc()
    in_maps = _make_in_maps(x, w_qkv, w_out, b_out)
    res = run_bass_kernel_spmd(nc, in_maps, list(range(N_CORES)), trace=trace)
    return _assemble(res.results), res


def kernel(x, w_qkv, w_out, b_out, f=32):
    out, _ = run(x, w_qkv, w_out, b_out, f=f, trace=False)
    return out
